# revision 1
# baseline (speedup 1.0000x reference)
"""BitLinear (RMSNorm + ternary-quantized linear) Trainium2 kernel.

Full-input contract: kernel(**inputs) takes the unsharded numpy inputs and
returns the full [B, S, DOUT] float32 output.

Strategy (column parallel over 8 NeuronCores):
  - Host: thr = mean(|w|) (computed with CPU jax to match the fp32 reduction
    order of the reference), ternarize w -> {-1,0,+1}, fold gamma in, cast
    to bf16 (exact for ternary values), lay out feature-major. x is cast to
    bf16 and repacked chunk-major ([c*128+p, k, m], 16KB contiguous DMA rows
    per 512-token chunk).
  - Device (per core, SPMD): weight shard [2048, 1024] bf16 resident in
    SBUF. Stream x in chunks of 512 tokens; sum(x^2) per token via one big
    ScalarE Square + a 4-deep VectorE tree-add over the k axis + a tiny
    bf16 ones-matmul for the cross-partition reduction;
    r = 1/sqrt(sum/DIN + eps) via ScalarE Sqrt + VectorE reciprocal
    (ScalarE Rsqrt is banned for accuracy). Main matmul: out[tok, n]
    accumulated over 16 K-tiles in PSUM (bf16 operands, fp32 accumulate,
    512-wide moving operand), evicted with tensor_scalar_mul by r.
  - Host gathers the 8 [M, 1024] shards along the feature axis and adds
    bias there (exact fp32; it is all-zeros for this problem).
Measured on the 8 axon trn2 cores: ~1.05 ms per-core HW exec, which is at
the machine's matmul roofline (4096 MMs x 512 cols at the ~2.0-2.3 GHz
observed PE clock). An fp8 hi/lo DoubleRow variant (use_fp8=True) was
implemented but DoubleRow gives no speedup on this hardware build, so
bf16 is the default.
"""

import numpy as np

B, S, DIN, DOUT = 4, 4096, 2048, 8192
M = B * S  # 16384
NCORES = 8
NSHARD = DOUT // NCORES  # 1024
P = 128
KT = DIN // P  # 16 k-tiles
MCH = 512  # tokens per chunk
G = MCH // P  # 4 groups of 128 tokens per chunk
H = NSHARD // 512  # 2 n-halves
EPS = float(np.finfo(np.float32).eps)

_CACHE = {}


def build_nc(m_tokens=M, n_shard=NSHARD, do_norm=True, do_mm=True, reps=1,
             use_fp8=False, defer_incs=False):
    # defer_incs batches per-MM PE-sem increments (~26ns serialized EVT_SEM
    # write each, ~100us total) onto the last MM of wait-free PE runs.
    # CLOSED as infeasible at this layer: a minimal 4-MM toy (3 deferred
    # incs, totals preserved, provably cycle-free) still deadlocks CoreSim,
    # identically whether the pass runs before or after bacc compile and
    # whether sync_info is mutated in place or rebuilt. Conclusion: the
    # executor gates per-instruction completion on precomputed per-
    # instruction tick values (rust-side vector clocks), not on the BIR
    # sync_info arithmetic, so increment batching must be done inside
    # Tile's sem-assignment (tile_sem_assignment / bass_rust), not by BIR
    # post-processing. Real HW might accept the batched stream, but
    # shipping a CoreSim-rejected program is not acceptable. Keep off.
    import concourse.bacc as bacc
    import concourse.mybir as mybir
    import concourse.tile as tile

    nch = m_tokens // MCH
    f32 = mybir.dt.float32
    bf16 = mybir.dt.bfloat16

    nc = bacc.Bacc("TRN2", target_bir_lowering=False, debug=False,
                   num_devices=NCORES)
    fp8 = mybir.dt.float8e4
    KP = KT // 2
    if use_fp8:
        # hi/lo e4m3 split of x; feature f = kp*256 + i*128 + p
        xhi_h = nc.dram_tensor("xhi", [(m_tokens // MCH) * P, KP, 2, MCH],
                               fp8, kind="ExternalInput")
        xlo_h = nc.dram_tensor("xlo", [(m_tokens // MCH) * P, KP, 2, MCH],
                               fp8, kind="ExternalInput")
        wt_h = nc.dram_tensor("wt", [P, KP, 2, n_shard], fp8,
                              kind="ExternalInput")
        xhi, xlo, wt = xhi_h.ap(), xlo_h.ap(), wt_h.ap()
    else:
        # chunk-major host layouts: xt[c*P+p, k, m] = x[c*MCH+m, k*P+p]
        # -> each chunk's DMA reads 128 partitions x 16KB contiguous rows.
        xt_h = nc.dram_tensor("xt", [(m_tokens // MCH) * P, KT, MCH], bf16,
                              kind="ExternalInput")
        # wt[p, k, n] = w_eff.T[k*P+p, n]
        wt_h = nc.dram_tensor("wt", [P, KT, n_shard], bf16,
                              kind="ExternalInput")
        xt = xt_h.ap()
        wt = wt_h.ap()
    out_h = nc.dram_tensor("out", [m_tokens, n_shard], f32,
                           kind="ExternalOutput")
    out = out_h.ap()

    Sqrt = mybir.ActivationFunctionType.Sqrt

    with tile.TileContext(nc) as tc:
        with (
            tc.tile_pool(name="const", bufs=1) as constp,
            tc.tile_pool(name="xin", bufs=2) as xin,
            tc.tile_pool(name="sq", bufs=3) as sqp,
            tc.tile_pool(name="acc", bufs=2) as accp,
            tc.tile_pool(name="nrm", bufs=2) as nrmp,
            tc.tile_pool(name="ev", bufs=4) as evp,
            tc.tile_pool(name="ps", bufs=5, space="PSUM") as psp,
            tc.tile_pool(name="psms", bufs=2, space="PSUM") as psmsp,
        ):
            # --- constants / weights resident in SBUF ---
            if use_fp8:
                w_sb = constp.tile([P, KP, 2, n_shard], fp8)
            else:
                w_sb = constp.tile([P, KT, n_shard], bf16)
            nc.sync.dma_start(w_sb[:], wt[:])
            ones_col = constp.tile([P, 1], bf16)
            nc.vector.memset(ones_col[:], 1.0)
            eps_col = constp.tile([P, 1], f32)
            nc.vector.memset(eps_col[:], EPS)

            import contextlib
            rep_ctx = (tc.For_i(0, reps, 1) if reps > 1
                       else contextlib.nullcontext())
            with rep_ctx:
              for c in range(nch):
                m0 = c * MCH
                if use_fp8:
                    x_hi = xin.tile([P, KP, 2, MCH], fp8, tag="xhi")
                    nc.sync.dma_start(x_hi[:], xhi[c * P:(c + 1) * P])
                    x_lo = xin.tile([P, KP, 2, MCH], fp8, tag="xlo")
                    nc.sync.dma_start(x_lo[:], xlo[c * P:(c + 1) * P])
                    sq_src = x_hi[:].rearrange("p k i m -> p (k i m)")
                else:
                    x_sb = xin.tile([P, KT, MCH], bf16, tag="x")
                    nc.sync.dma_start(x_sb[:], xt[c * P:(c + 1) * P, :, :])
                    sq_src = x_sb[:].rearrange("p k m -> p (k m)")

                r_sb = None
                if do_norm:
                    # sum of squares over features (partition dim spread over
                    # KT tiles): one big square on ScalarE, then a 4-deep
                    # in-place tree add over the k axis on VectorE.
                    # (fp8 path: squares from x_hi only; ms rel err ~1e-3)
                    sqf = sqp.tile([P, KT * MCH], mybir.dt.float32,
                                   tag="sqf")
                    nc.scalar.square(sqf[:], sq_src)
                    half = KT * MCH // 2
                    while half >= MCH:
                        nc.vector.tensor_add(sqf[:, :half], sqf[:, :half],
                                             sqf[:, half:2 * half])
                        half //= 2

                    # cross-partition sum per token group -> psum [128, G]
                    # (bf16 operands: fp32 self-loading matmuls trip a walrus
                    # sync-wait-slot limit; bf16 partials ~1e-4 rel on ms)
                    acc_bf = sqp.tile([P, MCH], bf16, tag="accbf")
                    nc.vector.tensor_copy(acc_bf[:], sqf[:, :MCH])
                    ps_ms = psmsp.tile([P, G], mybir.dt.float32, tag="ms")
                    for g in range(G):
                        nc.tensor.matmul(ps_ms[:, g:g + 1],
                                         acc_bf[:, g * P:(g + 1) * P],
                                         ones_col[:], start=True, stop=True)
                    # r = 1 / sqrt(sum/DIN + eps)
                    sqms = nrmp.tile([P, G], mybir.dt.float32, tag="sqms")
                    nc.scalar.activation(sqms[:], ps_ms[:], Sqrt,
                                         bias=eps_col[:], scale=1.0 / DIN)
                    r_sb = nrmp.tile([P, G], mybir.dt.float32, tag="r")
                    nc.vector.reciprocal(r_sb[:], sqms[:])

                if do_mm:
                    for g in range(G):
                        pss = [psp.tile([P, 512], mybir.dt.float32,
                                        tag="ps", name=f"ps{c}_{g}_{h}")
                               for h in range(H)]
                        # k outer, h inner: both matmuls of a k share the
                        # same stationary (x) tile
                        if use_fp8:
                            for xi, xx in enumerate((x_hi, x_lo)):
                                for kp in range(KP):
                                    for h in range(H):
                                        nc.tensor.matmul(
                                            pss[h][:],
                                            xx[:, kp, :,
                                               g * P:(g + 1) * P],
                                            w_sb[:, kp, :,
                                                 h * 512:(h + 1) * 512],
                                            start=(xi == 0 and kp == 0),
                                            stop=(xi == 1 and kp == KP - 1),
                                            perf_mode=(
                                                mybir.MatmulPerfMode
                                                .DoubleRow))
                        else:
                            for k in range(KT):
                                for h in range(H):
                                    nc.tensor.matmul(
                                        pss[h][:],
                                        x_sb[:, k, g * P:(g + 1) * P],
                                        w_sb[:, k, h * 512:(h + 1) * 512],
                                        start=(k == 0), stop=(k == KT - 1))
                        for h in range(H):
                            ev = evp.tile([P, 512], mybir.dt.float32,
                                          tag="ev")
                            if do_norm:
                                nc.vector.tensor_scalar_mul(
                                    ev[:], pss[h][:], r_sb[:, g:g + 1])
                            else:
                                nc.vector.tensor_copy(ev[:], pss[h][:])
                            nc.sync.dma_start(
                                out[m0 + g * P:m0 + (g + 1) * P,
                                    h * 512:(h + 1) * 512],
                                ev[:])
                elif do_norm:
                    # store r so the norm path isn't dead code
                    ev = evp.tile([P, G], mybir.dt.float32, tag="ev")
                    nc.vector.tensor_copy(ev[:], r_sb[:])
                    nc.sync.dma_start(out[m0:m0 + P, c * G:(c + 1) * G],
                                      ev[:])
    nc.compile()
    if defer_incs:
        # Must run AFTER bacc's compile: its passes
        # (move_matmul_waits_to_ldweights / generate_event_semaphores)
        # rewrite matmul sync_info and would drop the batched values.
        _defer_mm_incs(nc, mybir)
    return nc


def _defer_mm_incs(nc, mybir):
    """Batch per-matmul PE-sem increments onto the last matmul of each
    wait-free run of PE instructions. The PE proceeds unconditionally
    through such a run (no waits), so deferring increments within it only
    delays when other engines' `sem >= N` waits are satisfied — never a
    cycle — and totals are exactly preserved at every PE wait boundary.
    Saves the ~26ns serialized EVT_SEM write per intermediate matmul."""
    pe = mybir.EngineType.PE

    for b in nc.m.functions[0].blocks:
        run = []  # MMs in current wait-free PE run with a single sem-inc

        def flush():
            if len(run) > 1:
                sem_groups = {}
                for inst in run:
                    u = inst.sync_info.on_update[0]
                    sem_groups.setdefault(u.id, []).append(inst)
                for insts in sem_groups.values():
                    total = sum(i.sync_info.on_update[0].update_value
                                for i in insts)
                    for i in insts[:-1]:
                        i.sync_info = None
                    # nested update_value mutation is not seen by the rust
                    # executor; assign a freshly built SyncInfo instead
                    u = insts[-1].sync_info.on_update[0]
                    nu = type(u)(sync_type=u.sync_type, id=u.id,
                                 ant_name=u.ant_name,
                                 update_mode=u.update_mode,
                                 update_value=total,
                                 update_reg=u.update_reg)
                    insts[-1].sync_info = mybir.SyncInfo(
                        on_wait=[], on_update=[nu])
            run.clear()

        for inst in b.instructions:
            if getattr(inst, "engine", None) != pe:
                continue
            si = inst.sync_info
            has_wait = si is not None and bool(si.on_wait)
            if has_wait or not isinstance(inst, mybir.InstMatmult):
                if has_wait:
                    flush()
                continue
            if inst.start_tensor_calc:
                flush()
            if (si is not None and len(si.on_update) == 1
                    and si.on_update[0].update_mode == "sem-inc"):
                run.append(inst)
        flush()


USE_FP8 = False


def _host_prep_fp8(x, weight, bias, gamma):
    import jax
    import jax.numpy as jnp
    import ml_dtypes

    e4 = ml_dtypes.float8_e4m3
    KP = KT // 2
    w32 = np.asarray(weight, np.float32)
    with jax.default_device(jax.devices("cpu")[0]):
        thr = np.float32(jnp.mean(jnp.abs(jnp.asarray(w32))))
    wq = (np.sign(w32) * (np.abs(w32) > thr)).astype(np.float32)
    weff = wq * np.asarray(gamma, np.float32)[None, :]  # [DOUT, DIN]
    # feature f = kp*256 + i*128 + p; w8[p, kp, i, n] = weff.T[f, n]
    # (exact in e4m3 for ternary weights with gamma == 1)
    w8 = np.ascontiguousarray(
        weff.T.reshape(KP, 2, P, DOUT).transpose(2, 0, 1, 3)
    ).astype(e4)  # [P, KP, 2, DOUT]

    x32 = np.asarray(x, np.float32).reshape(M, DIN)
    hi = x32.astype(e4)
    lo = (x32 - hi.astype(np.float32)).astype(e4)

    def pack(a):
        return np.ascontiguousarray(
            a.reshape(M // MCH, MCH, KP, 2, P).transpose(0, 4, 2, 3, 1)
        ).reshape((M // MCH) * P, KP, 2, MCH)

    b32 = np.ascontiguousarray(np.asarray(bias, np.float32))
    return pack(hi), pack(lo), w8, b32


def _host_prep(x, weight, bias, gamma):
    import jax
    import jax.numpy as jnp
    import ml_dtypes

    w32 = np.asarray(weight, np.float32)
    try:
        # CPU jax reproduces the reference's fp32 reduction order bitwise;
        # ~2 weights sit within 1 ulp of thr, so the order matters.
        with jax.default_device(jax.devices("cpu")[0]):
            thr = np.float32(jnp.mean(jnp.abs(jnp.asarray(w32))))
    except Exception:
        thr = np.float32(np.mean(np.abs(w32)))
    wq = (np.sign(w32) * (np.abs(w32) > thr)).astype(np.float32)
    weff = wq * np.asarray(gamma, np.float32)[None, :]  # [DOUT, DIN]
    # chunk-major weight: wT[p, k, n] = weff.T[k*P+p, n], per full DOUT
    wT = np.ascontiguousarray(
        weff.T.reshape(KT, P, DOUT).transpose(1, 0, 2)
    ).astype(ml_dtypes.bfloat16)  # [P, KT, DOUT]

    # chunk-major x: xt[c*P+p, k, m] = x[c*MCH+m, k*P+p]
    x32 = np.asarray(x, np.float32).reshape(M, DIN)
    xb = x32.astype(ml_dtypes.bfloat16)
    xT = np.ascontiguousarray(
        xb.reshape(M // MCH, MCH, KT, P).transpose(0, 3, 2, 1)
    ).reshape((M // MCH) * P, KT, MCH)
    b32 = np.ascontiguousarray(np.asarray(bias, np.float32))
    return xT, wT, b32


def kernel(x, weight, bias, gamma):
    from concourse.bass_utils import run_bass_kernel_spmd

    if "nc" not in _CACHE:
        _CACHE["nc"] = build_nc(use_fp8=USE_FP8)
    nc = _CACHE["nc"]

    if USE_FP8:
        xhi, xlo, w8, b32 = _host_prep_fp8(x, weight, bias, gamma)
        in_maps = [
            {
                "xhi": xhi,
                "xlo": xlo,
                "wt": np.ascontiguousarray(
                    w8[:, :, :, c * NSHARD:(c + 1) * NSHARD]),
            }
            for c in range(NCORES)
        ]
    else:
        xT, wT, b32 = _host_prep(x, weight, bias, gamma)
        in_maps = [
            {
                "xt": xT,
                "wt": np.ascontiguousarray(
                    wT[:, :, c * NSHARD:(c + 1) * NSHARD]),
            }
            for c in range(NCORES)
        ]
    res = run_bass_kernel_spmd(nc, in_maps, core_ids=list(range(NCORES)))
    shards = [res.results[c]["out"] for c in range(NCORES)]
    full = np.concatenate(shards, axis=1)
    if np.any(b32):
        full += b32[None, :]
    return np.ascontiguousarray(
        full.reshape(B, S, DOUT).astype(np.float32, copy=False))



# revision 2
# speedup vs baseline: 2.7469x; 2.7469x over previous
"""BitLinear (RMSNorm + ternary linear) Trainium2 kernel, v2.

Contract: kernel(**inputs) takes full unsharded numpy inputs and returns
the full [B, S, DOUT] float32 output. Column-parallel over 8 cores.

Measured cost structure on these axon trn2 cores (microbenched):
  - every matmul slot costs ~325 ns = 512-cycle fill @2.4GHz + ~107 ns of
    unhidden LDWEIGHTS (128 stationary cols @1.2GHz) regardless of dtype;
    redundant/shared stationaries are NOT deduped, fp32r is no faster,
    int8/uint8 is rejected by the BIR verifier.
  - fp8e4 + DoubleRow contracts 2 k-tiles per slot: a true 2.02x.
  - single-stream e4m3 for all of x fails the 2e-2 gate (2.24e-2).

So v2 uses a hybrid: host pre-normalizes x (folds the RMSNorm scale in),
keeps EXACT=KT-CHEAP k-tiles of x in bf16 (1 slot per k-tile) and CHEAP
k-tiles in e4m3 fp8 processed pairwise with perf_mode=DoubleRow (1 slot
per 2 k-tiles). Ternary weights are exact in both dtypes. Slot count per
512-wide output block: 16 - CHEAP/2 (vs 16 for pure bf16). Accumulation
of both parts shares one PSUM bank; eviction is a plain copy.
"""

import numpy as np

B, S, DIN, DOUT = 4, 4096, 2048, 8192
M = B * S  # 16384
NCORES = 8
NSHARD = DOUT // NCORES  # 1024
P = 128
KT = DIN // P  # 16 k-tiles
MCH = 512  # tokens per chunk
G = MCH // P  # 4 stationary groups per chunk
H = NSHARD // 512  # 2 n-halves
CHEAP = 8  # k-tiles computed in fp8 DoubleRow (must be even)
EPS = float(np.finfo(np.float32).eps)

_CACHE = {}


def build_nc(m_tokens=M, n_shard=NSHARD, reps=1, cheap=CHEAP):
    import concourse.bacc as bacc
    import concourse.mybir as mybir
    import concourse.tile as tile
    import contextlib

    nch = m_tokens // MCH
    exact = KT - cheap
    cp = cheap // 2
    f32 = mybir.dt.float32
    bf16 = mybir.dt.bfloat16
    fp8 = mybir.dt.float8e4
    DR = mybir.MatmulPerfMode.DoubleRow

    nc = bacc.Bacc("TRN2", target_bir_lowering=False, debug=False,
                   num_devices=NCORES)

    # Host layouts (xn = pre-normalized x):
    #   xb[c*P+p, k, m]    = bf16(xn[c*MCH+m, k*P+p])            k<exact
    #   x8[c*P+p, kp, i, m]= e4m3(xn[c*MCH+m, exact*P + kp*256 + i*128 + p])
    #   wb[p, k, n]        = bf16(wq[n, k*P+p])                  k<exact
    #   w8[p, kp, i, n]    = e4m3(wq[n, exact*P + kp*256 + i*128 + p])
    xb_h = nc.dram_tensor("xb", [nch * P, exact, MCH], bf16,
                          kind="ExternalInput") if exact else None
    x8_h = nc.dram_tensor("x8", [nch * P, cp, 2, MCH], fp8,
                          kind="ExternalInput") if cp else None
    wb_h = nc.dram_tensor("wb", [P, exact, n_shard], bf16,
                          kind="ExternalInput") if exact else None
    w8_h = nc.dram_tensor("w8", [P, cp, 2, n_shard], fp8,
                          kind="ExternalInput") if cp else None
    out_h = nc.dram_tensor("out", [m_tokens, n_shard], f32,
                           kind="ExternalOutput")
    out = out_h.ap()

    with tile.TileContext(nc) as tc:
        with (
            tc.tile_pool(name="const", bufs=1) as constp,
            tc.tile_pool(name="xin", bufs=2) as xin,
            tc.tile_pool(name="ev", bufs=4) as evp,
            tc.tile_pool(name="ps", bufs=8, space="PSUM") as psp,
        ):
            if exact:
                wb_sb = constp.tile([P, exact, n_shard], bf16)
                nc.sync.dma_start(wb_sb[:], wb_h.ap()[:])
            if cp:
                w8_sb = constp.tile([P, cp, 2, n_shard], fp8)
                nc.sync.dma_start(w8_sb[:], w8_h.ap()[:])

            rep_ctx = (tc.For_i(0, reps, 1) if reps > 1
                       else contextlib.nullcontext())
            with rep_ctx:
                for c in range(nch):
                    m0 = c * MCH
                    if exact:
                        xb_sb = xin.tile([P, exact, MCH], bf16, tag="xb")
                        nc.sync.dma_start(
                            xb_sb[:], xb_h.ap()[c * P:(c + 1) * P])
                    if cp:
                        x8_sb = xin.tile([P, cp, 2, MCH], fp8, tag="x8")
                        nc.sync.dma_start(
                            x8_sb[:], x8_h.ap()[c * P:(c + 1) * P])
                    for g in range(G):
                        # k outer, h inner: consecutive matmuls alternate
                        # between the two PSUM banks, hiding the per-bank
                        # drain stall (~107 ns) behind the next fill.
                        pss = [psp.tile([P, 512], f32, tag="ps",
                                        name=f"ps{c}_{g}_{h}")
                               for h in range(H)]
                        for k in range(exact):
                            for h in range(H):
                                nc.tensor.matmul(
                                    pss[h][:],
                                    xb_sb[:, k, g * P:(g + 1) * P],
                                    wb_sb[:, k, h * 512:(h + 1) * 512],
                                    start=(k == 0),
                                    stop=(cp == 0 and k == exact - 1))
                        for kp in range(cp):
                            for h in range(H):
                                nc.tensor.matmul(
                                    pss[h][:],
                                    x8_sb[:, kp, :, g * P:(g + 1) * P],
                                    w8_sb[:, kp, :, h * 512:(h + 1) * 512],
                                    start=(exact == 0 and kp == 0),
                                    stop=(kp == cp - 1),
                                    perf_mode=DR)
                        for h in range(H):
                            ev = evp.tile([P, 512], f32, tag="ev")
                            nc.vector.tensor_copy(ev[:], pss[h][:])
                            nc.sync.dma_start(
                                out[m0 + g * P:m0 + (g + 1) * P,
                                    h * 512:(h + 1) * 512],
                                ev[:])
    nc.compile()
    return nc


def _host_prep(x, weight, bias, gamma, cheap=CHEAP):
    """Returns (xb, x8, wb, w8, b32) host arrays in device layouts.
    wb/w8 contain the FULL DOUT; caller shards along the n axis."""
    import jax
    import jax.numpy as jnp
    import ml_dtypes

    exact = KT - cheap
    cp = cheap // 2
    nb = exact * P

    w32 = np.asarray(weight, np.float32)
    try:
        # CPU jax reproduces the reference's fp32 reduction order bitwise;
        # ~2 weights sit within 1 ulp of thr, so the order matters.
        with jax.default_device(jax.devices("cpu")[0]):
            thr = np.float32(jnp.mean(jnp.abs(jnp.asarray(w32))))
    except Exception:
        thr = np.float32(np.mean(np.abs(w32)))
    wq = (np.sign(w32) * (np.abs(w32) > thr)).astype(np.float32)
    weff = wq * np.asarray(gamma, np.float32)[None, :]  # [DOUT, DIN]

    # pre-normalize x on host (fp32), matching the reference's rsqrt
    x32 = np.asarray(x, np.float32).reshape(M, DIN)
    ms = np.mean(x32 * x32, axis=1, keepdims=True, dtype=np.float32)
    r = (1.0 / np.sqrt(ms + EPS)).astype(np.float32)
    xn = x32 * r

    e4 = ml_dtypes.float8_e4m3

    xb = x8 = wb = w8 = None
    if exact:
        # xb[c*P+p, k, m] = xn[c*MCH+m, k*P+p]
        xb = np.ascontiguousarray(
            xn[:, :nb].astype(ml_dtypes.bfloat16)
            .reshape(M // MCH, MCH, exact, P).transpose(0, 3, 2, 1)
        ).reshape((M // MCH) * P, exact, MCH)
        # wb[p, k, n] = weff.T[k*P+p, n]
        wb = np.ascontiguousarray(
            weff[:, :nb].T.reshape(exact, P, DOUT).transpose(1, 0, 2)
        ).astype(ml_dtypes.bfloat16)
    if cp:
        xc = np.clip(xn[:, nb:], -240.0, 240.0).astype(e4)  # [M, cheap*P]
        # x8[c*P+p, kp, i, m] = xc[c*MCH+m, kp*256 + i*128 + p]
        x8 = np.ascontiguousarray(
            xc.reshape(M // MCH, MCH, cp, 2, P).transpose(0, 4, 2, 3, 1)
        ).reshape((M // MCH) * P, cp, 2, MCH)
        # w8[p, kp, i, n] = weff.T[nb + kp*256 + i*128 + p, n]
        w8 = np.ascontiguousarray(
            weff[:, nb:].T.reshape(cp, 2, P, DOUT).transpose(2, 0, 1, 3)
        ).astype(e4)
    b32 = np.ascontiguousarray(np.asarray(bias, np.float32))
    return xb, x8, wb, w8, b32


def _in_maps(xb, x8, wb, w8):
    maps = []
    for c in range(NCORES):
        m = {}
        if xb is not None:
            m["xb"] = xb
            m["wb"] = np.ascontiguousarray(
                wb[:, :, c * NSHARD:(c + 1) * NSHARD])
        if x8 is not None:
            m["x8"] = x8
            m["w8"] = np.ascontiguousarray(
                w8[:, :, :, c * NSHARD:(c + 1) * NSHARD])
        maps.append(m)
    return maps


def kernel(x, weight, bias, gamma):
    from concourse.bass_utils import run_bass_kernel_spmd

    if "nc" not in _CACHE:
        _CACHE["nc"] = build_nc()
    nc = _CACHE["nc"]

    xb, x8, wb, w8, b32 = _host_prep(x, weight, bias, gamma)
    in_maps = _in_maps(xb, x8, wb, w8)
    res = run_bass_kernel_spmd(nc, in_maps, core_ids=list(range(NCORES)))
    shards = [res.results[c]["out"] for c in range(NCORES)]
    full = np.concatenate(shards, axis=1)
    if np.any(b32):
        full += b32[None, :]
    return np.ascontiguousarray(
        full.reshape(B, S, DOUT).astype(np.float32, copy=False))
